# revision 18
# baseline (speedup 1.0000x reference)
"""Trainium2 Bass kernel for nn_BinaryLabelSoftRouter.

Reference computation (B=16, T=1024, D=2048, H=256, H2=128):
  base   = where(labels>0, [.25,.75], [.75,.25])            # (B,T,2)
  h1     = gelu(LN(x @ W1 + b1) * g1 + be1)                 # erf gelu
  h2     = gelu(LN(h1 @ W2 + b2) * g2 + be2)
  adj    = tanh(h2 @ W3 + b3) * 0.1
  p      = softmax((base + adj) / clip(temp, .1), -1)       # (B,T,2)
  out    = EMA over T (s_t = .9 s_{t-1} + .1 p_t, s_0 = p_0)

Sharding: data-parallel over batch, 2 rows per core x 8 cores.

Device-side structure (v2 -- all rewrites exact up to fp rounding):
  * X is pre-transposed AND pre-cast to bf16 on the host into the
    matmul lhsT layout [128, chunk, kc, 128], so the device does no
    x transposes and reads half the HBM bytes.
  * LN+gelu fused into ONE scalar-engine op per layer:
    gelu(LN(h)) = Gelu(h * rstd + (-mu * rstd)) with per-partition
    scale/bias APs.  All activation funcs used (Gelu / Tanh / Copy)
    live in the single act-table set `gelu_and_others` -> no table
    swaps.
  * softmax over 2 classes -> p1 = 0.5*tanh(d*inv_t/2) + 0.5 where d
    is the logit difference.  The affine 0.5x+0.5 commutes with the
    (linear) EMA, so the EMA runs on the single tanh column and the
    affine is applied once at output assembly.  EMA(const)=const
    holds for the truncated operator to ~1e-18.
  * EMA over each 128-step chunk is a lower-triangular [128,128]
    matmul; cross-chunk carry becomes rank-1 matmuls against the two
    previous chunks (0.9^256 ~ 1.8e-12 kills depth>=3).  Batched per
    row: 6 matmuls with overlapping PSUM accumulation ranges.
  * rstd = 1/sqrt(var+eps) via fast-inverse-sqrt (magic constant + 2
    Newton steps) on the vector engine, batched over 4 chunks.
  * PSUM packed to exactly 8 banks: mm1 pairs (2), mm2 quads (2),
    transpose quads (2), y/EMA row tiles (2).

Main matmuls run in bf16 (fp32 PSUM accumulation); EMA in fp32.
"""

import os
import numpy as np
import ml_dtypes

B, T, AD = 16, 1024, 2048
HID1, HID2 = 256, 128
NCORES = 8
B_LOC = B // NCORES            # 2 rows per core
CH_ROW = T // 128              # 8 chunks per row
CH = B_LOC * CH_ROW            # 16 chunks per core
GRP = 2                        # chunks per LN group (rsqrt batch)
KC = AD // 128                 # 16 contraction chunks for mm1
SM = 0.9
ADJ = 0.1
LN_EPS = 1e-5
MAGIC = 0x5f3759df - 0x00400000   # seed for rsqrt of v2 = v/2

_BF16 = ml_dtypes.bfloat16
_FP8 = ml_dtypes.float8_e4m3fn
FP8 = True            # mm1 in fp8e4m3 DoubleRow (W1 scaled by 256)
W1_SCALE = 256.0

_NC = {}
LAST_RESULTS = None


def _make_ema_mats():
    """EMA-as-matmul constants, all pre-transposed to lhsT layout [k, tau].

    s_c = A_loc @ p_c + 0.9^(tau+1) * s_{c-1}[127], and the carry expands
    into rank-1 matmuls against p_{c-1}, p_{c-2}: contributions beyond
    depth 2 carry a 0.9^256 ~ 1.8e-12 factor -> exactly zero in fp32.
    """
    tau = np.arange(128, dtype=np.float64)
    diff = tau[:, None] - tau[None, :]
    Am = np.where(diff >= 0, 0.1 * SM ** diff, 0.0)
    A0 = Am.copy()
    A0[:, 0] = SM ** tau
    dec = SM ** (tau + 1.0)          # 0.9^(tau+1)
    r1f = np.outer(A0[127, :], dec)  # [k, tau], carry from chunk 0
    r1m = np.outer(Am[127, :], dec)
    r2f = (SM ** 128) * r1f
    r2m = (SM ** 128) * r1m
    bfc = lambda a: np.ascontiguousarray(a.astype(_BF16))
    return {
        "a0t": bfc(A0.T), "amt": bfc(Am.T),
        "r1f": bfc(r1f), "r1m": bfc(r1m),
        "r2f": bfc(r2f), "r2m": bfc(r2m),
    }


def _build_nc(sim_gelu=False, triv1=True, triv2=True, trivb3=True, fp8=FP8):
    # trivN: layer-N has b==0, g==1, be==0 (true for this problem's
    # setup_inputs); the general path adds the bias matmul and two
    # affine ops before a plain (unfused) gelu.
    # sim_gelu: CoreSim has no Gelu LUT; substitute Tanh so the same
    # program structure can run under the simulator.
    import concourse.mybir as mybir
    import concourse.tile as tile
    from concourse import bacc

    f32 = mybir.dt.float32
    bf16 = mybir.dt.bfloat16
    i32 = mybir.dt.int32
    f8 = mybir.dt.float8e4
    xdt = f8 if fp8 else bf16
    AF = mybir.ActivationFunctionType
    OP = mybir.AluOpType
    GELU = AF.Tanh if sim_gelu else AF.Gelu

    nc = bacc.Bacc()

    # ---- DRAM parameters (per-core) ----
    xt_d = nc.declare_dram_parameter("xt", [128, CH, KC, 128], xdt,
                                     isOutput=False)
    w1_d = nc.declare_dram_parameter("w1", [128, KC, HID1], xdt,
                                     isOutput=False)
    w2_d = nc.declare_dram_parameter("w2", [128, 2, HID2], bf16,
                                     isOutput=False)
    w3_d = nc.declare_dram_parameter("w3", [128, 2], bf16, isOutput=False)
    lh_d = nc.declare_dram_parameter("lh", [128, CH], f32, isOutput=False)
    idb_d = nc.declare_dram_parameter("idbf", [128, 128], bf16,
                                      isOutput=False)
    magic_d = nc.declare_dram_parameter("magici", [128, 1], i32,
                                        isOutput=False)
    it2_d = nc.declare_dram_parameter("it2b", [128, 1], f32, isOutput=False)
    ema_d = {
        name: nc.declare_dram_parameter(name, [128, 128], bf16,
                                        isOutput=False)
        for name in ("a0t", "amt", "r1f", "r1m", "r2f", "r2m")
    }
    b1_d = nc.declare_dram_parameter("b1", [1, HID1], bf16, isOutput=False)
    b2_d = nc.declare_dram_parameter("b2", [1, HID2], bf16, isOutput=False)
    b3g_d = nc.declare_dram_parameter("b3g", [128, 2 * CH_ROW], f32,
                                      isOutput=False)
    g1_d = nc.declare_dram_parameter("g1bn", [128, HID1], f32,
                                     isOutput=False)
    be1_d = nc.declare_dram_parameter("be1b", [128, HID1], f32,
                                      isOutput=False)
    g2_d = nc.declare_dram_parameter("g2bn", [128, HID2], f32,
                                     isOutput=False)
    be2_d = nc.declare_dram_parameter("be2b", [128, HID2], f32,
                                      isOutput=False)
    ones_d = nc.declare_dram_parameter("ones1", [1, 128], bf16,
                                       isOutput=False)
    out_d = nc.declare_dram_parameter("out", [B_LOC, T, 2], f32,
                                      isOutput=True)

    with tile.TileContext(nc) as tc:
        with (
            tc.tile_pool(name="singles", bufs=1) as singles,
            tc.tile_pool(name="xio", bufs=4) as xio,
            tc.tile_pool(name="act", bufs=3) as act,
            tc.tile_pool(name="stat", bufs=3) as stat,
            tc.tile_pool(name="pm1", bufs=2, space="PSUM") as pm1,
            tc.tile_pool(name="pm2", bufs=2, space="PSUM") as pm2,
            tc.tile_pool(name="ptq", bufs=2, space="PSUM") as ptq,
            tc.tile_pool(name="pyr", bufs=2, space="PSUM") as pyr,
        ):
            # ---- resident tiles ----
            def load(name, shape, dt, src, gp=False):
                t = singles.tile(shape, dt, tag=name, name=name)
                if gp:
                    nc.gpsimd.dma_start(out=t[:], in_=src[:])
                else:
                    nc.sync.dma_start(t[:], src[:])
                return t


            pm1P = {}

            def issue_x(p):
                """DMA one pair of chunks of pre-transposed x."""
                xp = xio.tile([128, 2, KC, 128], xdt, tag="xp",
                              name=f"xp_{p}")
                nc.gpsimd.dma_start(out=xp[:], in_=xt_d[:, 2 * p:2 * p + 2])
                pm1P[("x", p)] = xp

            # Bulk constants go through the SWDGE (gpsimd) path -- the
            # HWDGE const queue moves large fragmented loads an order of
            # magnitude slower and would gate both startup and the tail.
            # w1 is split into quarters so mm1 can start after the first.
            w1_s = singles.tile([128, KC, HID1], xdt, tag="w1", name="w1")
            nc.gpsimd.dma_start(out=w1_s[:, 0:4, :], in_=w1_d[:, 0:4, :])
            xp0 = xio.tile([128, 2, KC, 128], xdt, tag="xp", name="xp_0")
            nc.gpsimd.dma_start(out=xp0[:, 0], in_=xt_d[:, 0])
            nc.gpsimd.dma_start(out=xp0[:, 1], in_=xt_d[:, 1])
            pm1P[("x", 0)] = xp0
            for wq in range(1, 4):
                nc.gpsimd.dma_start(out=w1_s[:, 4 * wq:4 * wq + 4, :],
                                    in_=w1_d[:, 4 * wq:4 * wq + 4, :])
            idb_s = load("idb", [128, 128], bf16, idb_d, gp=True)
            magic_s = load("magic", [128, 1], i32, magic_d)
            ones_s = (None if (triv1 and triv2)
                      else load("ones", [1, 128], bf16, ones_d))
            b1_s = None if triv1 else load("b1", [1, HID1], bf16, b1_d)

            def load_rest():
                nonlocal w2_s, w3_s, lh_s, it2_s, b2_s, b3g_s, \
                    g1_s, be1_s, g2_s, be2_s
                w2_s = load("w2", [128, 2, HID2], bf16, w2_d, gp=True)
                w3_s = load("w3", [128, 2], bf16, w3_d)
                lh_s = load("lh", [128, CH], f32, lh_d)
                it2_s = load("it2", [128, 1], f32, it2_d)
                b2_s = None if triv2 else load("b2", [1, HID2], bf16, b2_d)
                b3g_s = (None if trivb3
                         else load("b3g", [128, 2 * CH_ROW], f32, b3g_d))
                g1_s = be1_s = g2_s = be2_s = None
                if not triv1:
                    g1_s = load("g1", [128, HID1], f32, g1_d)
                    be1_s = load("be1", [128, HID1], f32, be1_d)
                if not triv2:
                    g2_s = load("g2", [128, HID2], f32, g2_d)
                    be2_s = load("be2", [128, HID2], f32, be2_d)

            def load_ema():
                nonlocal ema_s
                ema_s = {name: load(name, [128, 128], bf16, d, gp=True)
                         for name, d in ema_d.items()}

            w2_s = w3_s = lh_s = it2_s = None
            ema_s = None
            b2_s = b3g_s = g1_s = be1_s = g2_s = be2_s = None

            tc_full = singles.tile([128, CH], bf16)    # tanh cols for EMA
            sout = singles.tile([128, CH, 2], f32)     # final outputs

            def rsqrt_grp(var_ap, n, tagsuf):
                """positive 1/sqrt(var) via fast inverse sqrt + 1 Newton
                step.  eps is dropped (var >> eps here; ~3e-4 effect) and
                the v/2 halving is folded into MAGIC and the trailing
                constants.  Returns (rstd, y2n) with y2n = -2*rstd for the
                caller to fold signs/halves into."""
                ib = stat.tile([128, n], i32, tag="ib" + tagsuf)
                nc.vector.tensor_scalar(
                    out=ib[:], in0=var_ap.bitcast(i32), scalar1=1,
                    scalar2=None, op0=OP.logical_shift_right)
                y = stat.tile([128, n], f32, tag="y" + tagsuf)
                nc.vector.tensor_tensor(
                    out=y[:].bitcast(i32),
                    in0=magic_s[:].to_broadcast((128, n)), in1=ib[:],
                    op=OP.subtract)          # y0 ~ +rsqrt(v)
                p = stat.tile([128, n], f32, tag="p" + tagsuf)
                nc.vector.tensor_tensor(out=p[:], in0=y[:], in1=y[:],
                                        op=OP.mult)
                nc.vector.tensor_tensor(out=p[:], in0=p[:], in1=var_ap,
                                        op=OP.mult)     # p = v*y0^2
                nc.vector.scalar_tensor_tensor(
                    out=y[:], in0=p[:], scalar=3.0, in1=y[:],
                    op0=OP.subtract, op1=OP.mult)   # y2n = -2*rstd
                yp = stat.tile([128, n], f32, tag="yp" + tagsuf)
                nc.vector.tensor_scalar(
                    out=yp[:], in0=y[:], scalar1=-0.5, scalar2=None,
                    op0=OP.mult)                     # +rstd
                return yp, y

            # per-group state
            mv1G, rstd1G, nmr1G = {}, {}, {}
            mv2G, rstd2G, nmr2G = {}, {}, {}
            pm2Q, ptqQ = {}, {}
            pyR = {}


            def s1_chunk(c):
                """mm1 + LN1 stats (+ pairwise h1 copy to SBUF bf16)."""
                g, j = divmod(c, GRP)
                p, jp = divmod(c, 2)
                if j == 0:
                    mv1G[g] = stat.tile([128, GRP, 2], f32, tag="mv1",
                                        name=f"mv1_{g}")
                if jp == 0:
                    pm1P[p] = pm1.tile([128, 2, HID1], f32, tag="mm1",
                                       name=f"pm1_{p}")
                ph = pm1P[p]
                xp = pm1P[("x", p)]
                if fp8:
                    DR = mybir.MatmulPerfMode.DoubleRow
                    for k2 in range(KC // 2):
                        nc.tensor.matmul(
                            ph[:, jp, :], xp[:, jp, 2 * k2:2 * k2 + 2, :],
                            w1_s[:, 2 * k2:2 * k2 + 2, :],
                            start=(k2 == 0),
                            stop=(triv1 and k2 == KC // 2 - 1),
                            perf_mode=DR)
                else:
                    for k in range(KC):
                        nc.tensor.matmul(
                            ph[:, jp, :], xp[:, jp, k, :], w1_s[:, k, :],
                            start=(k == 0), stop=(triv1 and k == KC - 1))
                if not triv1:
                    nc.tensor.matmul(
                        ph[:, jp, :], ones_s[:], b1_s[:], start=False,
                        stop=True)
                st6 = stat.tile([128, 6], f32, tag="st6a")
                nc.vector.bn_stats(st6[:], ph[:, jp, :])
                nc.vector.bn_aggr(mv1G[g][:, j, :], st6[:])
                if jp == 1:
                    del pm1P[("x", p)]
                if j == GRP - 1 and jp == 1:
                    rpos, rneg = rsqrt_grp(mv1G[g][:, :, 1], GRP, "a")
                    rstd1G[g] = rpos
                    nm = stat.tile([128, GRP], f32, tag="nmr1")
                    nc.vector.scalar_tensor_tensor(
                        out=nm[:], in0=mv1G[g][:, :, 0], scalar=0.5,
                        in1=rneg[:], op0=OP.mult, op1=OP.mult)
                    nmr1G[g] = nm

            def s2_chunk(c):
                """fused LN1+gelu -> transpose -> mm2 -> LN2 stats."""
                g, j = divmod(c, GRP)
                p, jp = divmod(c, 2)
                q = g
                if j == 0:
                    mv2G[g] = stat.tile([128, GRP, 2], f32, tag="mv2",
                                        name=f"mv2_{g}")
                    pm2Q[q] = pm2.tile([128, GRP, HID2], f32, tag="mm2",
                                       name=f"pm2_{q}")
                    # one PSUM bank: pt1 of chunk j at cols 256j..256j+256;
                    # pt2 of chunk j reuses cols 256j..256j+128 (pt1 region
                    # is dead by stage 3).
                    ptqQ[q] = ptq.tile([128, 1024], bf16, tag="tq",
                                       name=f"ptq_{q}")
                ph1p = pm1P[p]
                h1g = act.tile([128, HID1], bf16, tag="h1g")
                if triv1:
                    nc.scalar.activation(
                        out=h1g[:], in_=ph1p[:, jp, :], func=GELU,
                        scale=rstd1G[g][:, j:j + 1],
                        bias=nmr1G[g][:, j:j + 1])
                else:
                    xn = act.tile([128, HID1], f32, tag="xn")
                    nc.vector.tensor_scalar(
                        out=xn[:], in0=ph1p[:, jp, :],
                        scalar1=mv1G[g][:, j, 0:1],
                        scalar2=rstd1G[g][:, j:j + 1],
                        op0=OP.subtract, op1=OP.mult)
                    nc.vector.scalar_tensor_tensor(
                        out=xn[:], in0=xn[:], scalar=1.0, in1=g1_s[:],
                        op0=OP.mult, op1=OP.mult)
                    nc.vector.tensor_tensor(
                        out=xn[:], in0=xn[:], in1=be1_s[:], op=OP.add)
                    nc.scalar.activation(out=h1g[:], in_=xn[:], func=GELU)
                if jp == 1:
                    del pm1P[p]
                pq = ptqQ[q]
                for k in range(2):
                    nc.tensor.transpose(
                        pq[:, 256 * j + 128 * k:256 * j + 128 * (k + 1)],
                        h1g[:, 128 * k:128 * (k + 1)], idb_s[:])
                h1t = act.tile([128, 2, 128], bf16, tag="h1t")
                nc.scalar.activation(
                    out=h1t[:], in_=pq[:, 256 * j:256 * (j + 1)],
                    func=AF.Copy)
                ph2 = pm2Q[q]
                for k in range(2):
                    nc.tensor.matmul(
                        ph2[:, j, :], h1t[:, k, :], w2_s[:, k, :],
                        start=(k == 0), stop=(triv2 and k == 1))
                if not triv2:
                    nc.tensor.matmul(
                        ph2[:, j, :], ones_s[:], b2_s[:], start=False,
                        stop=True)
                st6b = stat.tile([128, 6], f32, tag="st6b")
                nc.vector.bn_stats(st6b[:], ph2[:, j, :])
                nc.vector.bn_aggr(mv2G[g][:, j, :], st6b[:])
                if j == GRP - 1:
                    rpos2, rneg2 = rsqrt_grp(mv2G[g][:, :, 1], GRP, "b")
                    rstd2G[g] = rpos2
                    nm2 = stat.tile([128, GRP], f32, tag="nmr2")
                    nc.vector.scalar_tensor_tensor(
                        out=nm2[:], in0=mv2G[g][:, :, 0], scalar=0.5,
                        in1=rneg2[:], op0=OP.mult, op1=OP.mult)
                    nmr2G[g] = nm2

            def s3_chunk(c):
                """fused LN2+gelu -> transpose -> mm3."""
                g, j = divmod(c, GRP)
                q = g
                r, cc = divmod(c, CH_ROW)
                if cc == 0:
                    pyR[r] = pyr.tile([128, 3 * CH_ROW], f32, tag="yr",
                                      name=f"pyr_{r}")
                ph2 = pm2Q[q]
                h2g = act.tile([128, HID2], bf16, tag="h2g")
                if triv2:
                    nc.scalar.activation(
                        out=h2g[:], in_=ph2[:, j, :], func=GELU,
                        scale=rstd2G[g][:, j:j + 1],
                        bias=nmr2G[g][:, j:j + 1])
                else:
                    xn2 = act.tile([128, HID2], f32, tag="xn2")
                    nc.vector.tensor_scalar(
                        out=xn2[:], in0=ph2[:, j, :],
                        scalar1=mv2G[g][:, j, 0:1],
                        scalar2=rstd2G[g][:, j:j + 1],
                        op0=OP.subtract, op1=OP.mult)
                    nc.vector.scalar_tensor_tensor(
                        out=xn2[:], in0=xn2[:], scalar=1.0, in1=g2_s[:],
                        op0=OP.mult, op1=OP.mult)
                    nc.vector.tensor_tensor(
                        out=xn2[:], in0=xn2[:], in1=be2_s[:], op=OP.add)
                    nc.scalar.activation(out=h2g[:], in_=xn2[:], func=GELU)
                if j == GRP - 1:
                    del pm2Q[q]
                pq = ptqQ[q]
                nc.tensor.transpose(
                    pq[:, 256 * j:256 * j + 128], h2g[:], idb_s[:])
                h2t = act.tile([128, 128], bf16, tag="h2t")
                nc.scalar.activation(
                    out=h2t[:], in_=pq[:, 256 * j:256 * j + 128],
                    func=AF.Copy)
                if j == GRP - 1:
                    del ptqQ[q]
                nc.tensor.matmul(
                    pyR[r][:, 2 * cc:2 * cc + 2], h2t[:], w3_s[:],
                    start=True, stop=True, skip_group_check=True)

            def head_row(r):
                """tanh head + EMA + output assembly for one row."""
                py = pyR.pop(r)
                if not trivb3:
                    nc.vector.tensor_tensor(
                        out=py[:, :2 * CH_ROW], in0=py[:, :2 * CH_ROW],
                        in1=b3g_s[:], op=OP.add)
                th = stat.tile([128, CH_ROW, 2], f32, tag="th")
                nc.scalar.activation(
                    out=th[:].rearrange("p c n -> p (c n)"),
                    in_=py[:, :2 * CH_ROW], func=AF.Tanh)
                dcol = stat.tile([128, CH_ROW], f32, tag="dcol")
                nc.vector.tensor_tensor(
                    out=dcol[:], in0=th[:, :, 1], in1=th[:, :, 0],
                    op=OP.subtract)
                nc.vector.scalar_tensor_tensor(
                    out=dcol[:], in0=dcol[:], scalar=ADJ,
                    in1=lh_s[:, CH_ROW * r:CH_ROW * (r + 1)],
                    op0=OP.mult, op1=OP.add)
                tcs = tc_full[:, CH_ROW * r:CH_ROW * (r + 1)]
                nc.scalar.activation(out=tcs, in_=dcol[:], func=AF.Tanh,
                                     scale=it2_s[:])
                # EMA: 6 row-batched matmuls, overlapping accum ranges
                c0 = CH_ROW * r
                ps = py[:, 2 * CH_ROW:3 * CH_ROW]
                mms = [("a0t", c0, 1, 0, True),
                       ("amt", c0 + 1, 7, 1, True),
                       ("r1f", c0, 1, 1, False),
                       ("r1m", c0 + 1, 6, 2, False),
                       ("r2f", c0, 1, 2, False),
                       ("r2m", c0 + 1, 5, 3, False)]
                for i, (mat, cs, n, off, st) in enumerate(mms):
                    nc.tensor.matmul(
                        ps[:, off:off + n], ema_s[mat][:],
                        tc_full[:, cs:cs + n],
                        start=st, stop=(i == len(mms) - 1),
                        skip_group_check=True)
                # p1 = 0.5*E + 0.5 ; p0 = -0.5*E + 0.5
                so = sout[:, CH_ROW * r:CH_ROW * (r + 1), :]
                nc.vector.tensor_scalar(
                    out=so[:, :, 1], in0=ps[:], scalar1=0.5, scalar2=0.5,
                    op0=OP.mult, op1=OP.add)
                nc.vector.tensor_scalar(
                    out=so[:, :, 0], in0=ps[:], scalar1=-0.5, scalar2=0.5,
                    op0=OP.mult, op1=OP.add)
                nc.sync.dma_start(
                    out=out_d[r].rearrange("(c p) n -> p c n", p=128),
                    in_=so)

            # chunk-granular software pipeline
            D2, D3, DHD = 2, 5, 6
            issue_x(1)
            issue_x(2)
            first = True
            for t in range(0, CH + DHD + 1):
                if t % 2 == 0 and t // 2 + 3 < CH // 2:
                    issue_x(t // 2 + 3)
                if t < CH:
                    s1_chunk(t)
                if first:
                    load_rest()
                    first = False
                if t == 3:
                    load_ema()
                if 0 <= t - D3 < CH:
                    s3_chunk(t - D3)
                if 0 <= t - D2 < CH:
                    s2_chunk(t - D2)
                if t >= DHD and (t - DHD) % CH_ROW == CH_ROW - 1:
                    head_row((t - DHD) // CH_ROW)

    if not sim_gelu:
        nc.compile()
    return nc


def _get_nc(triv1=True, triv2=True, trivb3=True):
    key = (triv1, triv2, trivb3)
    if key not in _NC:
        _NC[key] = _build_nc(triv1=triv1, triv2=triv2, trivb3=trivb3)
    return _NC[key]


def _host_inputs(inputs):
    """Build the per-core input maps from the full problem inputs."""
    x = np.asarray(inputs["action_tokens"], np.float32)
    labels = np.asarray(inputs["critical_labels"]).astype(np.int32)
    W1 = np.asarray(inputs["W1"], np.float32)
    W2 = np.asarray(inputs["W2"], np.float32)
    W3 = np.asarray(inputs["W3"], np.float32)
    b1 = np.asarray(inputs["b1"], np.float32)
    b2 = np.asarray(inputs["b2"], np.float32)
    b3 = np.asarray(inputs["b3"], np.float32)
    g1 = np.asarray(inputs["g1"], np.float32)
    be1 = np.asarray(inputs["be1"], np.float32)
    g2 = np.asarray(inputs["g2"], np.float32)
    be2 = np.asarray(inputs["be2"], np.float32)
    temp = float(np.asarray(inputs["temperature"]))

    inv_t = np.float32(1.0 / max(temp, 0.1))
    ema = _make_ema_mats()

    # x -> fp8/bf16, then transpose to the mm1 lhsT layout:
    # xt[p, c, k, t] = x[row(c), 128*cc(c)+t, 128*k+p]
    xdt = _FP8 if FP8 else _BF16
    xb = x.astype(xdt)
    x5 = xb.reshape(B, CH_ROW, 128, KC, 128)          # [r, cc, t, k, p]
    xt_all = x5.transpose(4, 0, 1, 3, 2)              # [p, r, cc, k, t]

    w1h = (W1 * np.float32(W1_SCALE)) if FP8 else W1
    w1p = np.ascontiguousarray(
        w1h.reshape(KC, 128, HID1).transpose(1, 0, 2)).astype(xdt)
    w2p = np.ascontiguousarray(
        W2.reshape(2, 128, HID2).transpose(1, 0, 2)).astype(_BF16)
    w3p = W3.astype(_BF16)

    # lh[t, c] = labels[row(c), 128*cc(c)+t] - 0.5
    lh_all = (labels.reshape(B, CH_ROW, 128).transpose(2, 0, 1)
              .astype(np.float32) - np.float32(0.5))   # [t, r, cc]

    shared = {
        "w1": w1p,
        "w2": w2p,
        "w3": w3p,
        "b1": ((b1 * np.float32(W1_SCALE)) if FP8 else b1)
               .reshape(1, HID1).astype(_BF16),
        "b2": b2.reshape(1, HID2).astype(_BF16),
        "b3g": np.broadcast_to(np.tile(b3, CH_ROW), (128, 2 * CH_ROW))
                .astype(np.float32).copy(),
        "g1bn": np.broadcast_to(g1, (128, HID1)).copy(),
        "be1b": np.broadcast_to(be1, (128, HID1)).copy(),
        "g2bn": np.broadcast_to(g2, (128, HID2)).copy(),
        "be2b": np.broadcast_to(be2, (128, HID2)).copy(),
        **ema,
        "idbf": np.eye(128, dtype=_BF16),
        "ones1": np.ones((1, 128), dtype=_BF16),
        "magici": np.full((128, 1), MAGIC, np.int32),
        "it2b": np.full((128, 1), 0.5 * inv_t, np.float32),
    }

    in_maps = []
    for core in range(NCORES):
        r0 = core * B_LOC
        m = dict(shared)
        m["xt"] = np.ascontiguousarray(
            xt_all[:, r0:r0 + B_LOC]).reshape(128, CH, KC, 128)
        m["lh"] = np.ascontiguousarray(
            lh_all[:, r0:r0 + B_LOC]).reshape(128, CH)
        in_maps.append(m)
    return in_maps


def kernel(**inputs) -> np.ndarray:
    global LAST_RESULTS
    from concourse.bass_utils import run_bass_kernel_spmd

    triv1 = (not np.any(np.asarray(inputs["b1"]))
             and np.all(np.asarray(inputs["g1"]) == 1)
             and not np.any(np.asarray(inputs["be1"])))
    triv2 = (not np.any(np.asarray(inputs["b2"]))
             and np.all(np.asarray(inputs["g2"]) == 1)
             and not np.any(np.asarray(inputs["be2"])))
    trivb3 = not np.any(np.asarray(inputs["b3"]))
    nc = _get_nc(triv1, triv2, trivb3)
    in_maps = _host_inputs(inputs)
    trace = bool(int(os.environ.get("BLSR_TRACE", "0")))
    res = run_bass_kernel_spmd(
        nc, in_maps, list(range(NCORES)), trace=trace)
    LAST_RESULTS = res
    out = np.concatenate([res.results[i]["out"] for i in range(NCORES)],
                         axis=0)
    return out.astype(np.float32)


# revision 19
# speedup vs baseline: 1.0204x; 1.0204x over previous
"""Trainium2 Bass kernel for nn_BinaryLabelSoftRouter.

Reference computation (B=16, T=1024, D=2048, H=256, H2=128):
  base   = where(labels>0, [.25,.75], [.75,.25])            # (B,T,2)
  h1     = gelu(LN(x @ W1 + b1) * g1 + be1)                 # erf gelu
  h2     = gelu(LN(h1 @ W2 + b2) * g2 + be2)
  adj    = tanh(h2 @ W3 + b3) * 0.1
  p      = softmax((base + adj) / clip(temp, .1), -1)       # (B,T,2)
  out    = EMA over T (s_t = .9 s_{t-1} + .1 p_t, s_0 = p_0)

Sharding: data-parallel over batch, 2 rows per core x 8 cores.

Device-side structure (v2 -- all rewrites exact up to fp rounding):
  * X is pre-transposed AND pre-cast to bf16 on the host into the
    matmul lhsT layout [128, chunk, kc, 128], so the device does no
    x transposes and reads half the HBM bytes.
  * LN+gelu fused into ONE scalar-engine op per layer:
    gelu(LN(h)) = Gelu(h * rstd + (-mu * rstd)) with per-partition
    scale/bias APs.  All activation funcs used (Gelu / Tanh / Copy)
    live in the single act-table set `gelu_and_others` -> no table
    swaps.
  * softmax over 2 classes -> p1 = 0.5*tanh(d*inv_t/2) + 0.5 where d
    is the logit difference.  The affine 0.5x+0.5 commutes with the
    (linear) EMA, so the EMA runs on the single tanh column and the
    affine is applied once at output assembly.  EMA(const)=const
    holds for the truncated operator to ~1e-18.
  * EMA over each 128-step chunk is a lower-triangular [128,128]
    matmul; cross-chunk carry becomes rank-1 matmuls against the two
    previous chunks (0.9^256 ~ 1.8e-12 kills depth>=3).  Batched per
    row: 6 matmuls with overlapping PSUM accumulation ranges.
  * rstd = 1/sqrt(var+eps) via fast-inverse-sqrt (magic constant + 2
    Newton steps) on the vector engine, batched over 4 chunks.
  * PSUM packed to exactly 8 banks: mm1 pairs (2), mm2 quads (2),
    transpose quads (2), y/EMA row tiles (2).

Main matmuls run in bf16 (fp32 PSUM accumulation); EMA in fp32.
"""

import os
import numpy as np
import ml_dtypes

B, T, AD = 16, 1024, 2048
HID1, HID2 = 256, 128
NCORES = 8
B_LOC = B // NCORES            # 2 rows per core
CH_ROW = T // 128              # 8 chunks per row
CH = B_LOC * CH_ROW            # 16 chunks per core
GRP = 2                        # chunks per LN group (rsqrt batch)
KC = AD // 128                 # 16 contraction chunks for mm1
SM = 0.9
ADJ = 0.1
LN_EPS = 1e-5
MAGIC = 0x5f3759df - 0x00400000   # seed for rsqrt of v2 = v/2

_BF16 = ml_dtypes.bfloat16
_FP8 = ml_dtypes.float8_e4m3fn
FP8 = True            # mm1 in fp8e4m3 DoubleRow (W1 scaled by 256)
W1_SCALE = 256.0

_NC = {}
LAST_RESULTS = None


def _make_ema_mats():
    """EMA-as-matmul constants, all pre-transposed to lhsT layout [k, tau].

    s_c = A_loc @ p_c + 0.9^(tau+1) * s_{c-1}[127], and the carry expands
    into rank-1 matmuls against p_{c-1}, p_{c-2}: contributions beyond
    depth 2 carry a 0.9^256 ~ 1.8e-12 factor -> exactly zero in fp32.
    """
    tau = np.arange(128, dtype=np.float64)
    diff = tau[:, None] - tau[None, :]
    Am = np.where(diff >= 0, 0.1 * SM ** diff, 0.0)
    A0 = Am.copy()
    A0[:, 0] = SM ** tau
    dec = SM ** (tau + 1.0)          # 0.9^(tau+1)
    r1f = np.outer(A0[127, :], dec)  # [k, tau], carry from chunk 0
    r1m = np.outer(Am[127, :], dec)
    r2f = (SM ** 128) * r1f
    r2m = (SM ** 128) * r1m
    bfc = lambda a: np.ascontiguousarray(a.astype(_BF16))
    return {
        "a0t": bfc(A0.T), "amt": bfc(Am.T),
        "r1f": bfc(r1f), "r1m": bfc(r1m),
        "r2f": bfc(r2f), "r2m": bfc(r2m),
    }


def _build_nc(sim_gelu=False, triv1=True, triv2=True, trivb3=True, fp8=FP8):
    # trivN: layer-N has b==0, g==1, be==0 (true for this problem's
    # setup_inputs); the general path adds the bias matmul and two
    # affine ops before a plain (unfused) gelu.
    # sim_gelu: CoreSim has no Gelu LUT; substitute Tanh so the same
    # program structure can run under the simulator.
    import concourse.mybir as mybir
    import concourse.tile as tile
    from concourse import bacc

    f32 = mybir.dt.float32
    bf16 = mybir.dt.bfloat16
    i32 = mybir.dt.int32
    f8 = mybir.dt.float8e4
    xdt = f8 if fp8 else bf16
    AF = mybir.ActivationFunctionType
    OP = mybir.AluOpType
    GELU = AF.Tanh if sim_gelu else AF.Gelu

    nc = bacc.Bacc()

    # ---- DRAM parameters (per-core) ----
    xt_d = nc.declare_dram_parameter("xt", [128, CH, KC, 128], xdt,
                                     isOutput=False)
    w1_d = nc.declare_dram_parameter("w1", [128, KC, HID1], xdt,
                                     isOutput=False)
    w2_d = nc.declare_dram_parameter("w2", [128, 2, HID2], bf16,
                                     isOutput=False)
    w3_d = nc.declare_dram_parameter("w3", [128, 2], bf16, isOutput=False)
    lh_d = nc.declare_dram_parameter("lh", [128, CH], f32, isOutput=False)
    idb_d = nc.declare_dram_parameter("idbf", [128, 128], bf16,
                                      isOutput=False)
    magic_d = nc.declare_dram_parameter("magici", [128, 1], i32,
                                        isOutput=False)
    it2_d = nc.declare_dram_parameter("it2b", [128, 1], f32, isOutput=False)
    ema_d = {
        name: nc.declare_dram_parameter(name, [128, 128], bf16,
                                        isOutput=False)
        for name in ("a0t", "amt", "r1f", "r1m", "r2f", "r2m")
    }
    b1_d = nc.declare_dram_parameter("b1", [1, HID1], bf16, isOutput=False)
    b2_d = nc.declare_dram_parameter("b2", [1, HID2], bf16, isOutput=False)
    b3g_d = nc.declare_dram_parameter("b3g", [128, 2 * CH_ROW], f32,
                                      isOutput=False)
    g1_d = nc.declare_dram_parameter("g1bn", [128, HID1], f32,
                                     isOutput=False)
    be1_d = nc.declare_dram_parameter("be1b", [128, HID1], f32,
                                      isOutput=False)
    g2_d = nc.declare_dram_parameter("g2bn", [128, HID2], f32,
                                     isOutput=False)
    be2_d = nc.declare_dram_parameter("be2b", [128, HID2], f32,
                                      isOutput=False)
    ones_d = nc.declare_dram_parameter("ones1", [1, 128], bf16,
                                       isOutput=False)
    out_d = nc.declare_dram_parameter("out", [B_LOC, T, 2], f32,
                                      isOutput=True)

    with tile.TileContext(nc) as tc:
        with (
            tc.tile_pool(name="singles", bufs=1) as singles,
            tc.tile_pool(name="xio", bufs=4) as xio,
            tc.tile_pool(name="act", bufs=3) as act,
            tc.tile_pool(name="stat", bufs=3) as stat,
            tc.tile_pool(name="pm1", bufs=2, space="PSUM") as pm1,
            tc.tile_pool(name="pm2", bufs=2, space="PSUM") as pm2,
            tc.tile_pool(name="ptq", bufs=2, space="PSUM") as ptq,
            tc.tile_pool(name="pyr", bufs=2, space="PSUM") as pyr,
        ):
            # ---- resident tiles ----
            def load(name, shape, dt, src, gp=False):
                t = singles.tile(shape, dt, tag=name, name=name)
                if gp:
                    nc.gpsimd.dma_start(out=t[:], in_=src[:])
                else:
                    nc.sync.dma_start(t[:], src[:])
                return t


            pm1P = {}

            def issue_x(p):
                """DMA one pair of chunks of pre-transposed x."""
                xp = xio.tile([128, 2, KC, 128], xdt, tag="xp",
                              name=f"xp_{p}")
                nc.gpsimd.dma_start(out=xp[:], in_=xt_d[:, 2 * p:2 * p + 2])
                pm1P[("x", p)] = xp

            # Bulk constants go through the SWDGE (gpsimd) path -- the
            # HWDGE const queue moves large fragmented loads an order of
            # magnitude slower and would gate both startup and the tail.
            # w1 is split into quarters so mm1 can start after the first.
            w1_s = singles.tile([128, KC, HID1], xdt, tag="w1", name="w1")
            nc.gpsimd.dma_start(out=w1_s[:, 0:4, :], in_=w1_d[:, 0:4, :])
            xp0 = xio.tile([128, 2, KC, 128], xdt, tag="xp", name="xp_0")
            nc.gpsimd.dma_start(out=xp0[:, 0], in_=xt_d[:, 0])
            nc.gpsimd.dma_start(out=xp0[:, 1], in_=xt_d[:, 1])
            pm1P[("x", 0)] = xp0
            for wq in range(1, 4):
                nc.gpsimd.dma_start(out=w1_s[:, 4 * wq:4 * wq + 4, :],
                                    in_=w1_d[:, 4 * wq:4 * wq + 4, :])
            idb_s = load("idb", [128, 128], bf16, idb_d, gp=True)
            magic_s = load("magic", [128, 1], i32, magic_d)
            ones_s = (None if (triv1 and triv2)
                      else load("ones", [1, 128], bf16, ones_d))
            b1_s = None if triv1 else load("b1", [1, HID1], bf16, b1_d)

            def load_rest():
                nonlocal w2_s, w3_s, lh_s, it2_s, b2_s, b3g_s, \
                    g1_s, be1_s, g2_s, be2_s
                w2_s = load("w2", [128, 2, HID2], bf16, w2_d, gp=True)
                w3_s = load("w3", [128, 2], bf16, w3_d)
                lh_s = load("lh", [128, CH], f32, lh_d)
                it2_s = load("it2", [128, 1], f32, it2_d)
                b2_s = None if triv2 else load("b2", [1, HID2], bf16, b2_d)
                b3g_s = (None if trivb3
                         else load("b3g", [128, 2 * CH_ROW], f32, b3g_d))
                g1_s = be1_s = g2_s = be2_s = None
                if not triv1:
                    g1_s = load("g1", [128, HID1], f32, g1_d)
                    be1_s = load("be1", [128, HID1], f32, be1_d)
                if not triv2:
                    g2_s = load("g2", [128, HID2], f32, g2_d)
                    be2_s = load("be2", [128, HID2], f32, be2_d)

            def load_ema():
                nonlocal ema_s
                ema_s = {name: load(name, [128, 128], bf16, d, gp=True)
                         for name, d in ema_d.items()}

            w2_s = w3_s = lh_s = it2_s = None
            ema_s = None
            b2_s = b3g_s = g1_s = be1_s = g2_s = be2_s = None

            tc_full = singles.tile([128, CH], bf16)    # tanh cols for EMA
            sout = singles.tile([128, CH, 2], f32)     # final outputs

            def rsqrt_grp(var_ap, n, tagsuf):
                """positive 1/sqrt(var) via fast inverse sqrt + 1 Newton
                step.  eps is dropped (var >> eps here; ~3e-4 effect) and
                the v/2 halving is folded into MAGIC and the trailing
                constants.  Returns (rstd, y2n) with y2n = -2*rstd for the
                caller to fold signs/halves into."""
                ib = stat.tile([128, n], i32, tag="ib" + tagsuf)
                nc.vector.tensor_scalar(
                    out=ib[:], in0=var_ap.bitcast(i32), scalar1=1,
                    scalar2=None, op0=OP.logical_shift_right)
                y = stat.tile([128, n], f32, tag="y" + tagsuf)
                nc.vector.tensor_tensor(
                    out=y[:].bitcast(i32),
                    in0=magic_s[:].to_broadcast((128, n)), in1=ib[:],
                    op=OP.subtract)          # y0 ~ +rsqrt(v)
                p = stat.tile([128, n], f32, tag="p" + tagsuf)
                nc.vector.tensor_tensor(out=p[:], in0=y[:], in1=y[:],
                                        op=OP.mult)
                nc.vector.tensor_tensor(out=p[:], in0=p[:], in1=var_ap,
                                        op=OP.mult)     # p = v*y0^2
                nc.vector.scalar_tensor_tensor(
                    out=y[:], in0=p[:], scalar=3.0, in1=y[:],
                    op0=OP.subtract, op1=OP.mult)   # y2n = -2*rstd
                yp = stat.tile([128, n], f32, tag="yp" + tagsuf)
                nc.vector.tensor_scalar(
                    out=yp[:], in0=y[:], scalar1=-0.5, scalar2=None,
                    op0=OP.mult)                     # +rstd
                return yp, y

            # per-group state
            mv1G, rstd1G, nmr1G = {}, {}, {}
            mv2G, rstd2G, nmr2G = {}, {}, {}
            pm2Q, ptqQ = {}, {}
            pyR = {}


            def s1_chunk(c):
                """mm1 + LN1 stats (+ pairwise h1 copy to SBUF bf16)."""
                g, j = divmod(c, GRP)
                p, jp = divmod(c, 2)
                if j == 0:
                    mv1G[g] = stat.tile([128, GRP, 2], f32, tag="mv1",
                                        name=f"mv1_{g}")
                if jp == 0:
                    pm1P[p] = pm1.tile([128, 2, HID1], f32, tag="mm1",
                                       name=f"pm1_{p}")
                ph = pm1P[p]
                xp = pm1P[("x", p)]
                if fp8:
                    DR = mybir.MatmulPerfMode.DoubleRow
                    for k2 in range(KC // 2):
                        nc.tensor.matmul(
                            ph[:, jp, :], xp[:, jp, 2 * k2:2 * k2 + 2, :],
                            w1_s[:, 2 * k2:2 * k2 + 2, :],
                            start=(k2 == 0),
                            stop=(triv1 and k2 == KC // 2 - 1),
                            perf_mode=DR)
                else:
                    for k in range(KC):
                        nc.tensor.matmul(
                            ph[:, jp, :], xp[:, jp, k, :], w1_s[:, k, :],
                            start=(k == 0), stop=(triv1 and k == KC - 1))
                if not triv1:
                    nc.tensor.matmul(
                        ph[:, jp, :], ones_s[:], b1_s[:], start=False,
                        stop=True)
                st6 = stat.tile([128, 6], f32, tag="st6a")
                nc.vector.bn_stats(st6[:], ph[:, jp, :])
                nc.vector.bn_aggr(mv1G[g][:, j, :], st6[:])
                if jp == 1:
                    del pm1P[("x", p)]
                if j == GRP - 1 and jp == 1:
                    rpos, rneg = rsqrt_grp(mv1G[g][:, :, 1], GRP, "a")
                    rstd1G[g] = rpos
                    nm = stat.tile([128, GRP], f32, tag="nmr1")
                    nc.vector.scalar_tensor_tensor(
                        out=nm[:], in0=mv1G[g][:, :, 0], scalar=0.5,
                        in1=rneg[:], op0=OP.mult, op1=OP.mult)
                    nmr1G[g] = nm

            def s2_chunk(c):
                """fused LN1+gelu -> transpose -> mm2 -> LN2 stats."""
                g, j = divmod(c, GRP)
                p, jp = divmod(c, 2)
                q = g
                if j == 0:
                    mv2G[g] = stat.tile([128, GRP, 2], f32, tag="mv2",
                                        name=f"mv2_{g}")
                    pm2Q[q] = pm2.tile([128, GRP, HID2], f32, tag="mm2",
                                       name=f"pm2_{q}")
                    # one PSUM bank: pt1 of chunk j at cols 256j..256j+256;
                    # pt2 of chunk j reuses cols 256j..256j+128 (pt1 region
                    # is dead by stage 3).
                    ptqQ[q] = ptq.tile([128, 1024], bf16, tag="tq",
                                       name=f"ptq_{q}")
                ph1p = pm1P[p]
                h1g = act.tile([128, HID1], bf16, tag="h1g")
                if triv1:
                    nc.scalar.activation(
                        out=h1g[:], in_=ph1p[:, jp, :], func=GELU,
                        scale=rstd1G[g][:, j:j + 1],
                        bias=nmr1G[g][:, j:j + 1])
                else:
                    xn = act.tile([128, HID1], f32, tag="xn")
                    nc.vector.tensor_scalar(
                        out=xn[:], in0=ph1p[:, jp, :],
                        scalar1=mv1G[g][:, j, 0:1],
                        scalar2=rstd1G[g][:, j:j + 1],
                        op0=OP.subtract, op1=OP.mult)
                    nc.vector.scalar_tensor_tensor(
                        out=xn[:], in0=xn[:], scalar=1.0, in1=g1_s[:],
                        op0=OP.mult, op1=OP.mult)
                    nc.vector.tensor_tensor(
                        out=xn[:], in0=xn[:], in1=be1_s[:], op=OP.add)
                    nc.scalar.activation(out=h1g[:], in_=xn[:], func=GELU)
                if jp == 1:
                    del pm1P[p]
                pq = ptqQ[q]
                for k in range(2):
                    nc.tensor.transpose(
                        pq[:, 256 * j + 128 * k:256 * j + 128 * (k + 1)],
                        h1g[:, 128 * k:128 * (k + 1)], idb_s[:])
                h1t = act.tile([128, 2, 128], bf16, tag="h1t")
                nc.scalar.activation(
                    out=h1t[:], in_=pq[:, 256 * j:256 * (j + 1)],
                    func=AF.Copy)
                ph2 = pm2Q[q]
                for k in range(2):
                    nc.tensor.matmul(
                        ph2[:, j, :], h1t[:, k, :], w2_s[:, k, :],
                        start=(k == 0), stop=(triv2 and k == 1))
                if not triv2:
                    nc.tensor.matmul(
                        ph2[:, j, :], ones_s[:], b2_s[:], start=False,
                        stop=True)
                st6b = stat.tile([128, 6], f32, tag="st6b")
                nc.vector.bn_stats(st6b[:], ph2[:, j, :])
                nc.vector.bn_aggr(mv2G[g][:, j, :], st6b[:])
                if j == GRP - 1:
                    rpos2, rneg2 = rsqrt_grp(mv2G[g][:, :, 1], GRP, "b")
                    rstd2G[g] = rpos2
                    nm2 = stat.tile([128, GRP], f32, tag="nmr2")
                    nc.vector.scalar_tensor_tensor(
                        out=nm2[:], in0=mv2G[g][:, :, 0], scalar=0.5,
                        in1=rneg2[:], op0=OP.mult, op1=OP.mult)
                    nmr2G[g] = nm2

            def s3_chunk(c):
                """fused LN2+gelu -> transpose -> mm3."""
                g, j = divmod(c, GRP)
                q = g
                r, cc = divmod(c, CH_ROW)
                if cc == 0:
                    pyR[r] = pyr.tile([128, 3 * CH_ROW], f32, tag="yr",
                                      name=f"pyr_{r}")
                ph2 = pm2Q[q]
                h2g = act.tile([128, HID2], bf16, tag="h2g")
                if triv2:
                    nc.scalar.activation(
                        out=h2g[:], in_=ph2[:, j, :], func=GELU,
                        scale=rstd2G[g][:, j:j + 1],
                        bias=nmr2G[g][:, j:j + 1])
                else:
                    xn2 = act.tile([128, HID2], f32, tag="xn2")
                    nc.vector.tensor_scalar(
                        out=xn2[:], in0=ph2[:, j, :],
                        scalar1=mv2G[g][:, j, 0:1],
                        scalar2=rstd2G[g][:, j:j + 1],
                        op0=OP.subtract, op1=OP.mult)
                    nc.vector.scalar_tensor_tensor(
                        out=xn2[:], in0=xn2[:], scalar=1.0, in1=g2_s[:],
                        op0=OP.mult, op1=OP.mult)
                    nc.vector.tensor_tensor(
                        out=xn2[:], in0=xn2[:], in1=be2_s[:], op=OP.add)
                    nc.scalar.activation(out=h2g[:], in_=xn2[:], func=GELU)
                if j == GRP - 1:
                    del pm2Q[q]
                pq = ptqQ[q]
                nc.tensor.transpose(
                    pq[:, 256 * j:256 * j + 128], h2g[:], idb_s[:])
                h2t = act.tile([128, 128], bf16, tag="h2t")
                nc.scalar.activation(
                    out=h2t[:], in_=pq[:, 256 * j:256 * j + 128],
                    func=AF.Copy)
                if j == GRP - 1:
                    del ptqQ[q]
                nc.tensor.matmul(
                    pyR[r][:, 2 * cc:2 * cc + 2], h2t[:], w3_s[:],
                    start=True, stop=True, skip_group_check=True)

            def head_row(r):
                """tanh head + EMA + output assembly for one row."""
                py = pyR.pop(r)
                if not trivb3:
                    nc.vector.tensor_tensor(
                        out=py[:, :2 * CH_ROW], in0=py[:, :2 * CH_ROW],
                        in1=b3g_s[:], op=OP.add)
                th = stat.tile([128, CH_ROW, 2], f32, tag="th")
                nc.scalar.activation(
                    out=th[:].rearrange("p c n -> p (c n)"),
                    in_=py[:, :2 * CH_ROW], func=AF.Tanh)
                dcol = stat.tile([128, CH_ROW], f32, tag="dcol")
                nc.vector.tensor_tensor(
                    out=dcol[:], in0=th[:, :, 1], in1=th[:, :, 0],
                    op=OP.subtract)
                nc.vector.scalar_tensor_tensor(
                    out=dcol[:], in0=dcol[:], scalar=ADJ,
                    in1=lh_s[:, CH_ROW * r:CH_ROW * (r + 1)],
                    op0=OP.mult, op1=OP.add)
                tcs = tc_full[:, CH_ROW * r:CH_ROW * (r + 1)]
                nc.scalar.activation(out=tcs, in_=dcol[:], func=AF.Tanh,
                                     scale=it2_s[:])
                # EMA: 6 row-batched matmuls, overlapping accum ranges
                c0 = CH_ROW * r
                ps = py[:, 2 * CH_ROW:3 * CH_ROW]
                mms = [("a0t", c0, 1, 0, True),
                       ("amt", c0 + 1, 7, 1, True),
                       ("r1f", c0, 1, 1, False),
                       ("r1m", c0 + 1, 6, 2, False),
                       ("r2f", c0, 1, 2, False),
                       ("r2m", c0 + 1, 5, 3, False)]
                for i, (mat, cs, n, off, st) in enumerate(mms):
                    nc.tensor.matmul(
                        ps[:, off:off + n], ema_s[mat][:],
                        tc_full[:, cs:cs + n],
                        start=st, stop=(i == len(mms) - 1),
                        skip_group_check=True)
                # p1 = 0.5*E + 0.5 ; p0 = -0.5*E + 0.5
                so = sout[:, CH_ROW * r:CH_ROW * (r + 1), :]
                nc.vector.tensor_scalar(
                    out=so[:, :, 1], in0=ps[:], scalar1=0.5, scalar2=0.5,
                    op0=OP.mult, op1=OP.add)
                nc.vector.tensor_scalar(
                    out=so[:, :, 0], in0=ps[:], scalar1=-0.5, scalar2=0.5,
                    op0=OP.mult, op1=OP.add)
                nc.sync.dma_start(
                    out=out_d[r].rearrange("(c p) n -> p c n", p=128),
                    in_=so)

            # chunk-granular software pipeline
            D2, D3, DHD = 2, 4, 5
            issue_x(1)
            issue_x(2)
            first = True
            for t in range(0, CH + DHD + 1):
                if t % 2 == 0 and t // 2 + 3 < CH // 2:
                    issue_x(t // 2 + 3)
                if t < CH:
                    s1_chunk(t)
                if first:
                    load_rest()
                    first = False
                if t == 3:
                    load_ema()
                if 0 <= t - D2 < CH:
                    s2_chunk(t - D2)
                if 0 <= t - D3 < CH:
                    s3_chunk(t - D3)
                if t >= DHD and (t - DHD) % CH_ROW == CH_ROW - 1:
                    head_row((t - DHD) // CH_ROW)

    if not sim_gelu:
        nc.compile()
    return nc


def _get_nc(triv1=True, triv2=True, trivb3=True):
    key = (triv1, triv2, trivb3)
    if key not in _NC:
        _NC[key] = _build_nc(triv1=triv1, triv2=triv2, trivb3=trivb3)
    return _NC[key]


def _host_inputs(inputs):
    """Build the per-core input maps from the full problem inputs."""
    x = np.asarray(inputs["action_tokens"], np.float32)
    labels = np.asarray(inputs["critical_labels"]).astype(np.int32)
    W1 = np.asarray(inputs["W1"], np.float32)
    W2 = np.asarray(inputs["W2"], np.float32)
    W3 = np.asarray(inputs["W3"], np.float32)
    b1 = np.asarray(inputs["b1"], np.float32)
    b2 = np.asarray(inputs["b2"], np.float32)
    b3 = np.asarray(inputs["b3"], np.float32)
    g1 = np.asarray(inputs["g1"], np.float32)
    be1 = np.asarray(inputs["be1"], np.float32)
    g2 = np.asarray(inputs["g2"], np.float32)
    be2 = np.asarray(inputs["be2"], np.float32)
    temp = float(np.asarray(inputs["temperature"]))

    inv_t = np.float32(1.0 / max(temp, 0.1))
    ema = _make_ema_mats()

    # x -> fp8/bf16, then transpose to the mm1 lhsT layout:
    # xt[p, c, k, t] = x[row(c), 128*cc(c)+t, 128*k+p]
    xdt = _FP8 if FP8 else _BF16
    xb = x.astype(xdt)
    x5 = xb.reshape(B, CH_ROW, 128, KC, 128)          # [r, cc, t, k, p]
    xt_all = x5.transpose(4, 0, 1, 3, 2)              # [p, r, cc, k, t]

    w1h = (W1 * np.float32(W1_SCALE)) if FP8 else W1
    w1p = np.ascontiguousarray(
        w1h.reshape(KC, 128, HID1).transpose(1, 0, 2)).astype(xdt)
    w2p = np.ascontiguousarray(
        W2.reshape(2, 128, HID2).transpose(1, 0, 2)).astype(_BF16)
    w3p = W3.astype(_BF16)

    # lh[t, c] = labels[row(c), 128*cc(c)+t] - 0.5
    lh_all = (labels.reshape(B, CH_ROW, 128).transpose(2, 0, 1)
              .astype(np.float32) - np.float32(0.5))   # [t, r, cc]

    shared = {
        "w1": w1p,
        "w2": w2p,
        "w3": w3p,
        "b1": ((b1 * np.float32(W1_SCALE)) if FP8 else b1)
               .reshape(1, HID1).astype(_BF16),
        "b2": b2.reshape(1, HID2).astype(_BF16),
        "b3g": np.broadcast_to(np.tile(b3, CH_ROW), (128, 2 * CH_ROW))
                .astype(np.float32).copy(),
        "g1bn": np.broadcast_to(g1, (128, HID1)).copy(),
        "be1b": np.broadcast_to(be1, (128, HID1)).copy(),
        "g2bn": np.broadcast_to(g2, (128, HID2)).copy(),
        "be2b": np.broadcast_to(be2, (128, HID2)).copy(),
        **ema,
        "idbf": np.eye(128, dtype=_BF16),
        "ones1": np.ones((1, 128), dtype=_BF16),
        "magici": np.full((128, 1), MAGIC, np.int32),
        "it2b": np.full((128, 1), 0.5 * inv_t, np.float32),
    }

    in_maps = []
    for core in range(NCORES):
        r0 = core * B_LOC
        m = dict(shared)
        m["xt"] = np.ascontiguousarray(
            xt_all[:, r0:r0 + B_LOC]).reshape(128, CH, KC, 128)
        m["lh"] = np.ascontiguousarray(
            lh_all[:, r0:r0 + B_LOC]).reshape(128, CH)
        in_maps.append(m)
    return in_maps


def kernel(**inputs) -> np.ndarray:
    global LAST_RESULTS
    from concourse.bass_utils import run_bass_kernel_spmd

    triv1 = (not np.any(np.asarray(inputs["b1"]))
             and np.all(np.asarray(inputs["g1"]) == 1)
             and not np.any(np.asarray(inputs["be1"])))
    triv2 = (not np.any(np.asarray(inputs["b2"]))
             and np.all(np.asarray(inputs["g2"]) == 1)
             and not np.any(np.asarray(inputs["be2"])))
    trivb3 = not np.any(np.asarray(inputs["b3"]))
    nc = _get_nc(triv1, triv2, trivb3)
    in_maps = _host_inputs(inputs)
    trace = bool(int(os.environ.get("BLSR_TRACE", "0")))
    res = run_bass_kernel_spmd(
        nc, in_maps, list(range(NCORES)), trace=trace)
    LAST_RESULTS = res
    out = np.concatenate([res.results[i]["out"] for i in range(NCORES)],
                         axis=0)
    return out.astype(np.float32)
